# revision 21
# baseline (speedup 1.0000x reference)
# Trainium2 Bass kernel for nn_MinLoss_15229954032079.
#
# Math: loss = sum_b sum_s dist(p[b,s], g[b,match(b,s)]) / B, where
# dist is the euclidean distance between flattened [T*D] source signals
# and match is a greedy bipartite assignment on the [S,S] distance matrix.
#
# All pairwise distances derive from the 8x8 Gram matrix of the 8 flattened
# source vectors (4 prediction sources + 4 ground-truth sources) per batch:
#   d2[s,t] = G[s,s] + G[4+t,4+t] - 2*G[s,4+t]
#
# Strategy (one NeuronCore per batch element, 8 cores):
#   - Stream p[b], g[b] (16.8 MB each, f32) into SBUF in 8 windows via
#     contiguous HWDGE DMAs (one per source-tensor j per window).
#   - DVE/ACT copies cast f32 -> bf16 while shuffling each source's
#     columns into a blocked layout: column r*128 + j*16 + u, so every
#     matmul operand is a contiguous 128-column slice (walrus requires
#     single-free-dim matmul APs).
#   - For each 128-column block, accumulate PSUM += block^T @ block on
#     the PE. The [128,128] PSUM holds, at entry (16j+u, 16j'+u), partial
#     dot products of sources j,j' — summing the 16 u-diagonals on the
#     host yields the exact 8x8 Gram.
#   - Tiny [4,4] greedy matching + final scalar reduction on host.

import numpy as np

B, T, S, D = 8, 4096, 4, 257
NCORES = 8
NW = 16           # number of windows
TW = T // NW      # 256 time steps per window
TI = TW // 128    # 2 time steps per partition per window
CS = TW * D // 128  # 514 data columns per source-tensor per window
NJ = 8            # 4 pred sources + 4 gt sources
DL = 16 // TI     # d-values per (j, block): block cols = DL*TI = 16 per j
NBLK = 256 // DL  # full matmul blocks per window
WCOLS = NJ * CS   # body (128*NBLK) + tail (NJ*TI) columns
PSB = NJ * TI     # tail psum dim

_cached_nc = None


def _build_nc():
    import concourse.bacc as bacc
    import concourse.tile as tile
    from concourse import mybir

    nc = bacc.Bacc("TRN2", target_bir_lowering=False, debug=False, num_swdge_queues=2)
    p_dram = nc.dram_tensor("p", [T, S, D], mybir.dt.float32, kind="ExternalInput")
    g_dram = nc.dram_tensor("g", [T, S, D], mybir.dt.float32, kind="ExternalInput")
    gram_dram = nc.dram_tensor(
        "gram", [128, 128], mybir.dt.float32, kind="ExternalOutput"
    )
    gram2_dram = nc.dram_tensor(
        "gram2", [PSB, PSB], mybir.dt.float32, kind="ExternalOutput"
    )

    with tile.TileContext(nc) as tc:
        with (
            tc.tile_pool(name="slab", bufs=4) as fpool,
            tc.tile_pool(name="blk16", bufs=3) as bpool,
            tc.tile_pool(name="psum", bufs=1, space="PSUM") as ppool,
            tc.tile_pool(name="out", bufs=1) as opool,
        ):
            psa = ppool.tile([128, 128], mybir.dt.float32)
            psb = ppool.tile([PSB, PSB], mybir.dt.float32)
            # [T,S,D] -> [w, partition, ti, s, d]: partition p covers times
            # w*TW + p*TI + ti. One DMA per (window, tensor): the source is
            # fully contiguous per partition (TI*4*257 elems = 16 KB runs).
            p_view = p_dram.ap().rearrange(
                "(w p ti) s d -> w p ti s d", w=NW, p=128, ti=TI
            )
            g_view = g_dram.ap().rearrange(
                "(w p ti) s d -> w p ti s d", w=NW, p=128, ti=TI
            )

            n_mm = NW * NBLK
            mm_i = 0
            HALF = TI * S * D  # 4112 cols per tensor in raw HBM order
            for w in range(NW):
                # slab holds the window in raw HBM order: [p-tensor | g-tensor],
                # each column (ti, s, d) -> ti*1028 + s*257 + d. The DMA is a
                # plain contiguous copy (16.4 KB per partition) that also
                # casts f32 -> bf16 (SWDGE path).
                fsl = fpool.tile([128, 2 * HALF], mybir.dt.bfloat16)
                nc.gpsimd.dma_start(out=fsl[:, 0:HALF], in_=p_view[w])
                nc.gpsimd.dma_start(out=fsl[:, HALF : 2 * HALF], in_=g_view[w])

                wb = bpool.tile([128, WCOLS], mybir.dt.bfloat16)
                # per-source element order is d-major: q = (d, ti), so the
                # NBLK body blocks (16 cols = DL d x TI ti per j) are exactly
                # rectangular; the leftover d=256 gives TI tail cols per j.
                # body blocked col: r*128 + j*16 + dl*TI + ti
                wv = wb[:, 0 : 128 * NBLK].rearrange(
                    "p (r j dl ti) -> p j r dl ti", j=NJ, dl=DL, ti=TI
                )
                for j in range(NJ):
                    # p-sources shuffle+cast on DVE, g-sources on ACT: each
                    # copy waits on exactly one DMA; matmuls wait on both
                    # engines (Bacc's generate_event_semaphores legalizes
                    # the multi-wait).
                    off = 0 if j < 4 else HALF
                    half = fsl[:, off : off + HALF].rearrange(
                        "p (ti c) -> p ti c", ti=TI
                    )
                    srcj = half[:, :, (j % 4) * D : (j % 4 + 1) * D]  # [p, ti, d]
                    body = srcj[:, :, 0:256].rearrange(
                        "p ti (dg dl) -> p dg dl ti", dl=DL
                    )
                    tail = srcj[:, :, 256]  # [p, ti]
                    nc.vector.tensor_copy(wv[:, j], body)
                    nc.vector.tensor_copy(
                        wb[:, 128 * NBLK + TI * j : 128 * NBLK + TI * (j + 1)], tail
                    )

                for r in range(NBLK):
                    blk = wb[:, 128 * r : 128 * (r + 1)]
                    nc.tensor.matmul(
                        psa[:],
                        blk,
                        blk,
                        start=(mm_i == 0),
                        stop=(mm_i == n_mm - 1),
                    )
                    mm_i += 1
                tblk = wb[:, 128 * NBLK : 128 * NBLK + PSB]
                nc.tensor.matmul(
                    psb[:],
                    tblk,
                    tblk,
                    start=(w == 0),
                    stop=(w == NW - 1),
                )

            outt = opool.tile([128, 128], mybir.dt.float32)
            outt2 = opool.tile([PSB, PSB], mybir.dt.float32)
            nc.vector.tensor_copy(outt[:], psa[:])
            nc.vector.tensor_copy(outt2[:], psb[:])
            nc.sync.dma_start(out=gram_dram.ap(), in_=outt[:])
            nc.sync.dma_start(out=gram2_dram.ap(), in_=outt2[:])
    nc.compile()
    return nc


def _greedy_match_np(d):
    # replicate reference._greedy_match: repeated global argmin with
    # row/col masking; np.argmin matches jnp.argmin tie-breaking (first).
    s = d.shape[0]
    dm = d.astype(np.float32).copy()
    matches = np.zeros(s, np.int32)
    for _ in range(s):
        m = int(np.argmin(dm.reshape(-1)))
        r, c = divmod(m, s)
        matches[r] = c
        dm[r, :] = np.inf
        dm[:, c] = np.inf
    return matches


def _loss_from_gram(psa_list):
    total = 0.0
    for psa, psb in psa_list:
        # G8[j,j'] = sum_u psa[16j+u, 16j'+u] + sum_u psb[4j+u, 4j'+u]
        g8 = np.einsum("juku->jk", psa.reshape(8, 16, 8, 16).astype(np.float64))
        g8 += np.einsum("juku->jk", psb.reshape(8, TI, 8, TI).astype(np.float64))
        pn = np.diag(g8)[:4]
        gn = np.diag(g8)[4:]
        cr = g8[:4, 4:]
        d2 = pn[:, None] + gn[None, :] - 2.0 * cr
        dists = np.sqrt(np.maximum(d2, 0.0)).astype(np.float32)
        matches = _greedy_match_np(dists)
        total += float(dists[np.arange(4), matches].astype(np.float64).sum())
    return np.float32(total / B)


def kernel(**inputs):
    global _cached_nc
    preds = np.ascontiguousarray(inputs["predictions"], dtype=np.float32)
    gts = np.ascontiguousarray(inputs["ground_truths"], dtype=np.float32)
    assert preds.shape == (B, T, S, D) and gts.shape == (B, T, S, D)

    if _cached_nc is None:
        _cached_nc = _build_nc()
    nc = _cached_nc

    from concourse.bass_utils import run_bass_kernel_spmd

    in_maps = [{"p": preds[b], "g": gts[b]} for b in range(B)]
    res = run_bass_kernel_spmd(nc, in_maps, list(range(NCORES)))
    psa_list = [(res.results[b]["gram"], res.results[b]["gram2"]) for b in range(B)]
    return _loss_from_gram(psa_list)


# revision 22
# speedup vs baseline: 1.1102x; 1.1102x over previous
# Trainium2 Bass kernel for nn_MinLoss_15229954032079.
#
# Math: loss = sum_b sum_s dist(p[b,s], g[b,match(b,s)]) / B, where
# dist is the euclidean distance between flattened [T*D] source signals
# and match is a greedy bipartite assignment on the [S,S] distance matrix.
#
# All pairwise distances derive from the 8x8 Gram matrix of the 8 flattened
# source vectors (4 prediction sources + 4 ground-truth sources) per batch:
#   d2[s,t] = G[s,s] + G[4+t,4+t] - 2*G[s,4+t]
#
# Strategy (one NeuronCore per batch element, 8 cores):
#   - Stream p[b], g[b] (16.8 MB each, f32) into SBUF in 8 windows via
#     contiguous HWDGE DMAs (one per source-tensor j per window).
#   - DVE/ACT copies cast f32 -> bf16 while shuffling each source's
#     columns into a blocked layout: column r*128 + j*16 + u, so every
#     matmul operand is a contiguous 128-column slice (walrus requires
#     single-free-dim matmul APs).
#   - For each 128-column block, accumulate PSUM += block^T @ block on
#     the PE. The [128,128] PSUM holds, at entry (16j+u, 16j'+u), partial
#     dot products of sources j,j' — summing the 16 u-diagonals on the
#     host yields the exact 8x8 Gram.
#   - Tiny [4,4] greedy matching + final scalar reduction on host.

import numpy as np

B, T, S, D = 8, 4096, 4, 257
NCORES = 8
NW = 8            # number of windows
TW = T // NW      # 512 time steps per window
TI = TW // 128    # 4 time steps per partition per window
CS = TW * D // 128  # 1028 data columns per source-tensor per window
NJ = 8            # 4 pred sources + 4 gt sources
NBLK = TI * 16    # full matmul blocks per window: r = (ti, dg)
WCOLS = NJ * CS   # body (128*NBLK) + tail (NJ*TI) columns
PSB = NJ * TI     # tail psum dim

_cached_nc = None


def _build_nc():
    import concourse.bacc as bacc
    import concourse.tile as tile
    from concourse import mybir

    nc = bacc.Bacc("TRN2", target_bir_lowering=False, debug=False, num_swdge_queues=2)
    p_dram = nc.dram_tensor("p", [T, S, D], mybir.dt.float32, kind="ExternalInput")
    g_dram = nc.dram_tensor("g", [T, S, D], mybir.dt.float32, kind="ExternalInput")
    gram_dram = nc.dram_tensor(
        "gram", [128, 128], mybir.dt.float32, kind="ExternalOutput"
    )
    gram2_dram = nc.dram_tensor(
        "gram2", [PSB, PSB], mybir.dt.float32, kind="ExternalOutput"
    )

    with tile.TileContext(nc) as tc:
        with (
            tc.tile_pool(name="slab", bufs=4) as fpool,
            tc.tile_pool(name="blk16", bufs=3) as bpool,
            tc.tile_pool(name="psum", bufs=1, space="PSUM") as ppool,
            tc.tile_pool(name="out", bufs=1) as opool,
        ):
            psa = ppool.tile([128, 128], mybir.dt.float32)
            psb = ppool.tile([PSB, PSB], mybir.dt.float32)
            # [T,S,D] -> [w, partition, ti, s, d]: partition p covers times
            # w*TW + p*TI + ti. One DMA per (window, tensor): the source is
            # fully contiguous per partition (TI*4*257 elems = 16 KB runs).
            p_view = p_dram.ap().rearrange(
                "(w p ti) s d -> w p ti s d", w=NW, p=128, ti=TI
            )
            g_view = g_dram.ap().rearrange(
                "(w p ti) s d -> w p ti s d", w=NW, p=128, ti=TI
            )

            n_mm = NW * NBLK
            mm_i = 0
            HALF = TI * S * D  # 4112 cols per tensor in raw HBM order
            for w in range(NW):
                # slab holds the window in raw HBM order: [p-tensor | g-tensor],
                # each column (ti, s, d) -> ti*1028 + s*257 + d. The DMA is a
                # plain contiguous copy (16.4 KB per partition) that also
                # casts f32 -> bf16 (SWDGE path).
                fsl = fpool.tile([128, 2 * HALF], mybir.dt.bfloat16)
                nc.gpsimd.dma_start(out=fsl[:, 0:HALF], in_=p_view[w])
                nc.gpsimd.dma_start(out=fsl[:, HALF : 2 * HALF], in_=g_view[w])

                wb = bpool.tile([128, WCOLS], mybir.dt.bfloat16)
                # per-source element order: q = (ti, dg, dl) — each block
                # is one ti and 16 consecutive d's per j, so copies move
                # 16-element contiguous runs on both sides. The leftover
                # d=256 gives TI tail cols per j.
                # body blocked col: (ti*16+dg)*128 + j*16 + dl
                wv = wb[:, 0 : 128 * NBLK].rearrange(
                    "p (ti dg j dl) -> p j ti dg dl", ti=TI, dg=16, j=NJ, dl=16
                )
                for j in range(NJ):
                    # p-sources shuffle+cast on DVE, g-sources on ACT: each
                    # copy waits on exactly one DMA; matmuls wait on both
                    # engines (Bacc's generate_event_semaphores legalizes
                    # the multi-wait).
                    off = 0 if j < 4 else HALF
                    half = fsl[:, off : off + HALF].rearrange(
                        "p (ti c) -> p ti c", ti=TI
                    )
                    srcj = half[:, :, (j % 4) * D : (j % 4 + 1) * D]  # [p, ti, d]
                    body = srcj[:, :, 0:256].rearrange(
                        "p ti (dg dl) -> p ti dg dl", dl=16
                    )
                    tail = srcj[:, :, 256]  # [p, ti]
                    nc.vector.tensor_copy(wv[:, j], body)
                    nc.vector.tensor_copy(
                        wb[:, 128 * NBLK + TI * j : 128 * NBLK + TI * (j + 1)], tail
                    )

                for r in range(NBLK):
                    blk = wb[:, 128 * r : 128 * (r + 1)]
                    nc.tensor.matmul(
                        psa[:],
                        blk,
                        blk,
                        start=(mm_i == 0),
                        stop=(mm_i == n_mm - 1),
                    )
                    mm_i += 1
                tblk = wb[:, 128 * NBLK : 128 * NBLK + PSB]
                nc.tensor.matmul(
                    psb[:],
                    tblk,
                    tblk,
                    start=(w == 0),
                    stop=(w == NW - 1),
                )

            outt = opool.tile([128, 128], mybir.dt.float32)
            outt2 = opool.tile([PSB, PSB], mybir.dt.float32)
            nc.vector.tensor_copy(outt[:], psa[:])
            nc.vector.tensor_copy(outt2[:], psb[:])
            nc.sync.dma_start(out=gram_dram.ap(), in_=outt[:])
            nc.sync.dma_start(out=gram2_dram.ap(), in_=outt2[:])
    nc.compile()
    return nc


def _greedy_match_np(d):
    # replicate reference._greedy_match: repeated global argmin with
    # row/col masking; np.argmin matches jnp.argmin tie-breaking (first).
    s = d.shape[0]
    dm = d.astype(np.float32).copy()
    matches = np.zeros(s, np.int32)
    for _ in range(s):
        m = int(np.argmin(dm.reshape(-1)))
        r, c = divmod(m, s)
        matches[r] = c
        dm[r, :] = np.inf
        dm[:, c] = np.inf
    return matches


def _loss_from_gram(psa_list):
    total = 0.0
    for psa, psb in psa_list:
        # G8[j,j'] = sum_u psa[16j+u, 16j'+u] + sum_u psb[4j+u, 4j'+u]
        g8 = np.einsum("juku->jk", psa.reshape(8, 16, 8, 16).astype(np.float64))
        g8 += np.einsum("juku->jk", psb.reshape(8, TI, 8, TI).astype(np.float64))
        pn = np.diag(g8)[:4]
        gn = np.diag(g8)[4:]
        cr = g8[:4, 4:]
        d2 = pn[:, None] + gn[None, :] - 2.0 * cr
        dists = np.sqrt(np.maximum(d2, 0.0)).astype(np.float32)
        matches = _greedy_match_np(dists)
        total += float(dists[np.arange(4), matches].astype(np.float64).sum())
    return np.float32(total / B)


def kernel(**inputs):
    global _cached_nc
    preds = np.ascontiguousarray(inputs["predictions"], dtype=np.float32)
    gts = np.ascontiguousarray(inputs["ground_truths"], dtype=np.float32)
    assert preds.shape == (B, T, S, D) and gts.shape == (B, T, S, D)

    if _cached_nc is None:
        _cached_nc = _build_nc()
    nc = _cached_nc

    from concourse.bass_utils import run_bass_kernel_spmd

    in_maps = [{"p": preds[b], "g": gts[b]} for b in range(B)]
    res = run_bass_kernel_spmd(nc, in_maps, list(range(NCORES)))
    psa_list = [(res.results[b]["gram"], res.results[b]["gram2"]) for b in range(B)]
    return _loss_from_gram(psa_list)


# revision 26
# speedup vs baseline: 1.1672x; 1.0514x over previous
# Trainium2 Bass kernel for nn_MinLoss_15229954032079.
#
# Math: loss = sum_b sum_s dist(p[b,s], g[b,match(b,s)]) / B, where
# dist is the euclidean distance between flattened [T*D] source signals
# and match is a greedy bipartite assignment on the [S,S] distance matrix.
#
# All pairwise distances derive from the 8x8 Gram matrix of the 8 flattened
# source vectors (4 prediction sources + 4 ground-truth sources) per batch:
#   d2[s,t] = G[s,s] + G[4+t,4+t] - 2*G[s,4+t]
#
# Strategy (one NeuronCore per batch element, 8 cores):
#   - Stream p[b], g[b] (16.8 MB each, f32) into SBUF in 8 windows via
#     contiguous HWDGE DMAs (one per source-tensor j per window).
#   - DVE/ACT copies cast f32 -> bf16 while shuffling each source's
#     columns into a blocked layout: column r*128 + j*16 + u, so every
#     matmul operand is a contiguous 128-column slice (walrus requires
#     single-free-dim matmul APs).
#   - For each 128-column block, accumulate PSUM += block^T @ block on
#     the PE. The [128,128] PSUM holds, at entry (16j+u, 16j'+u), partial
#     dot products of sources j,j' — summing the 16 u-diagonals on the
#     host yields the exact 8x8 Gram.
#   - Tiny [4,4] greedy matching + final scalar reduction on host.

import numpy as np

B, T, S, D = 8, 4096, 4, 257
NCORES = 8
NW = 8            # number of windows
TW = T // NW      # 512 time steps per window
TI = TW // 128    # 4 time steps per partition per window
CS = TW * D // 128  # 1028 data columns per source-tensor per window
NJ = 8            # 4 pred sources + 4 gt sources
NBLK = TI * 16    # full matmul blocks per window: r = (ti, dg)
WCOLS = NJ * CS   # body (128*NBLK) + tail (NJ*TI) columns
PSB = NJ * TI     # tail psum dim

_cached_nc = None


def _light_drain_and_barrier(self, tick_clock, wait_clock):
    # Replaces TileContext._drain_and_barrier: keep the drain + one
    # all-engine barrier, but skip the per-semaphore clear pass and the
    # second barrier (~6 us). Safe here because every kernel() invocation
    # executes a freshly loaded NEFF, so semaphores start from zero and
    # don't need to be restored for a re-run.
    from concourse.vector_clock import ScopedClock

    drain_inst = self.nc.sync.drain()
    wait_clock.add_sem_waits(
        drain_inst.ins, ScopedClock({None: tick_clock.global_clock})
    )
    self.nc.all_engine_barrier()
    popped = self.nc._tile_sem_poison_stack.pop()
    assert popped is self._sem_poison


def _build_nc():
    import concourse.bacc as bacc
    import concourse.tile as tile
    from concourse import mybir

    nc = bacc.Bacc("TRN2", target_bir_lowering=False, debug=False, num_swdge_queues=2)
    p_dram = nc.dram_tensor("p", [T, S, D], mybir.dt.float32, kind="ExternalInput")
    g_dram = nc.dram_tensor("g", [T, S, D], mybir.dt.float32, kind="ExternalInput")
    gram_dram = nc.dram_tensor(
        "gram", [128, 128], mybir.dt.float32, kind="ExternalOutput"
    )
    gram2_dram = nc.dram_tensor(
        "gram2", [PSB, PSB], mybir.dt.float32, kind="ExternalOutput"
    )

    tile.TileContext._drain_and_barrier = _light_drain_and_barrier

    with tile.TileContext(nc) as tc:
        with (
            tc.tile_pool(name="slab", bufs=4) as fpool,
            tc.tile_pool(name="slab0", bufs=1) as f0pool,
            tc.tile_pool(name="blk16", bufs=3) as bpool,
            tc.tile_pool(name="psum", bufs=1, space="PSUM") as ppool,
            tc.tile_pool(name="out", bufs=1) as opool,
        ):
            psa = ppool.tile([128, 128], mybir.dt.float32)
            psb = ppool.tile([PSB, PSB], mybir.dt.float32)
            # [T,S,D] -> [w, partition, ti, s, d]: partition p covers times
            # w*TW + p*TI + ti. One DMA per (window, tensor): the source is
            # fully contiguous per partition (TI*4*257 elems = 16 KB runs).
            p_view = p_dram.ap().rearrange(
                "(w p ti) s d -> w p ti s d", w=NW, p=128, ti=TI
            )
            g_view = g_dram.ap().rearrange(
                "(w p ti) s d -> w p ti s d", w=NW, p=128, ti=TI
            )

            n_mm = NW * NBLK
            mm_i = 0
            HALF = TI * S * D  # 4112 cols per tensor in raw HBM order
            for w in range(NW):
                # slab holds the window in raw HBM order: [p-tensor | g-tensor],
                # each column (ti, s, d) -> ti*1028 + s*257 + d. The DMA is a
                # plain contiguous copy (16.4 KB per partition) that also
                # casts f32 -> bf16 (SWDGE path). Window 0 instead goes
                # through HWDGE in f32 (its first transfer starts ~7 us
                # earlier than the SWDGE path); its copies do the cast.
                if w == 0:
                    fsl = f0pool.tile([128, 2 * HALF], mybir.dt.float32)
                    nc.sync.dma_start(out=fsl[:, 0:HALF], in_=p_view[w])
                    nc.sync.dma_start(out=fsl[:, HALF : 2 * HALF], in_=g_view[w])
                else:
                    fsl = fpool.tile([128, 2 * HALF], mybir.dt.bfloat16)
                    nc.gpsimd.dma_start(out=fsl[:, 0:HALF], in_=p_view[w])
                    nc.gpsimd.dma_start(out=fsl[:, HALF : 2 * HALF], in_=g_view[w])

                wb = bpool.tile([128, WCOLS], mybir.dt.bfloat16)
                # per-source element order: q = (ti, dg, dl) — each block
                # is one ti and 16 consecutive d's per j, so copies move
                # 16-element contiguous runs on both sides. The leftover
                # d=256 gives TI tail cols per j.
                # body blocked col: (ti*16+dg)*128 + j*16 + dl
                wv = wb[:, 0 : 128 * NBLK].rearrange(
                    "p (ti dg j dl) -> p j ti dg dl", ti=TI, dg=16, j=NJ, dl=16
                )
                for j in range(NJ):
                    # p-sources shuffle+cast on DVE, g-sources on ACT: each
                    # copy waits on exactly one DMA; matmuls wait on both
                    # engines (Bacc's generate_event_semaphores legalizes
                    # the multi-wait).
                    off = 0 if j < 4 else HALF
                    half = fsl[:, off : off + HALF].rearrange(
                        "p (ti c) -> p ti c", ti=TI
                    )
                    srcj = half[:, :, (j % 4) * D : (j % 4 + 1) * D]  # [p, ti, d]
                    body = srcj[:, :, 0:256].rearrange(
                        "p ti (dg dl) -> p ti dg dl", dl=16
                    )
                    tail = srcj[:, :, 256]  # [p, ti]
                    nc.vector.tensor_copy(wv[:, j], body)
                    nc.vector.tensor_copy(
                        wb[:, 128 * NBLK + TI * j : 128 * NBLK + TI * (j + 1)], tail
                    )

                for r in range(NBLK):
                    blk = wb[:, 128 * r : 128 * (r + 1)]
                    nc.tensor.matmul(
                        psa[:],
                        blk,
                        blk,
                        start=(mm_i == 0),
                        stop=(mm_i == n_mm - 1),
                    )
                    mm_i += 1
                tblk = wb[:, 128 * NBLK : 128 * NBLK + PSB]
                nc.tensor.matmul(
                    psb[:],
                    tblk,
                    tblk,
                    start=(w == 0),
                    stop=(w == NW - 1),
                )

            outt = opool.tile([128, 128], mybir.dt.float32)
            outt2 = opool.tile([PSB, PSB], mybir.dt.float32)
            nc.vector.tensor_copy(outt[:], psa[:])
            nc.vector.tensor_copy(outt2[:], psb[:])
            nc.sync.dma_start(out=gram_dram.ap(), in_=outt[:])
            nc.sync.dma_start(out=gram2_dram.ap(), in_=outt2[:])
    nc.compile()
    return nc


def _greedy_match_np(d):
    # replicate reference._greedy_match: repeated global argmin with
    # row/col masking; np.argmin matches jnp.argmin tie-breaking (first).
    s = d.shape[0]
    dm = d.astype(np.float32).copy()
    matches = np.zeros(s, np.int32)
    for _ in range(s):
        m = int(np.argmin(dm.reshape(-1)))
        r, c = divmod(m, s)
        matches[r] = c
        dm[r, :] = np.inf
        dm[:, c] = np.inf
    return matches


def _loss_from_gram(psa_list):
    total = 0.0
    for psa, psb in psa_list:
        # G8[j,j'] = sum_u psa[16j+u, 16j'+u] + sum_u psb[4j+u, 4j'+u]
        g8 = np.einsum("juku->jk", psa.reshape(8, 16, 8, 16).astype(np.float64))
        g8 += np.einsum("juku->jk", psb.reshape(8, TI, 8, TI).astype(np.float64))
        pn = np.diag(g8)[:4]
        gn = np.diag(g8)[4:]
        cr = g8[:4, 4:]
        d2 = pn[:, None] + gn[None, :] - 2.0 * cr
        dists = np.sqrt(np.maximum(d2, 0.0)).astype(np.float32)
        matches = _greedy_match_np(dists)
        total += float(dists[np.arange(4), matches].astype(np.float64).sum())
    return np.float32(total / B)


def kernel(**inputs):
    global _cached_nc
    preds = np.ascontiguousarray(inputs["predictions"], dtype=np.float32)
    gts = np.ascontiguousarray(inputs["ground_truths"], dtype=np.float32)
    assert preds.shape == (B, T, S, D) and gts.shape == (B, T, S, D)

    if _cached_nc is None:
        _cached_nc = _build_nc()
    nc = _cached_nc

    from concourse.bass_utils import run_bass_kernel_spmd

    in_maps = [{"p": preds[b], "g": gts[b]} for b in range(B)]
    res = run_bass_kernel_spmd(nc, in_maps, list(range(NCORES)))
    psa_list = [(res.results[b]["gram"], res.results[b]["gram2"]) for b in range(B)]
    return _loss_from_gram(psa_list)
